# revision 7
# baseline (speedup 1.0000x reference)
"""Bahdanau-additive attention scorer on 8 TRN2 NeuronCores.

reference:
  wq = context @ Wc.T            (B, CTX, D)
  uh = queries @ Wq.T + bq       (B, QRS, D)
  scores[b,c,q] = sum_h v[h] * tanh(wq[b,c,h] + uh[b,q,h])
  return scores.reshape(B, QRS, CTX)     # flat view of (B, CTX, QRS)

Sharding: over (batch, query): core k handles batch k//4, queries
(k%4)*64 ... +64, with the full 1024-row context of its batch (context
replicated across the 4 cores of a batch, weights everywhere).

Inputs are pre-split on host into fp16 (hi, lo) pairs so the device matmuls
get ~fp32 accuracy at fp16 PE speed (products keep hi*hi + hi*lo + lo*hi).

Device layout: hidden dim h on partitions (2 tiles of 128 = "m" halves).
  prologue (PE): wqT[h,c] = Wc @ ctx.T (2 x [128,1024], fp16 out)
                 uhT[h,q] = Wq @ q.T + bq (2 x [128,64], fp32)
  main loop over 16 groups of 8 units (unit = (m, q), m-major):
    S[:, j*1024:+1024] = wqT_m + uhT_m[:, q]   DVE tensor_scalar_add (fp16)
    T = tanh(S)  fp16                          ACT, [128, 8192] per instr
    per 128-col chunk: psum[:, col:col+2] = T_chunk.T @ (v_hi|v_lo)  PE N=2
  epilogue (overlapped): DVE sums the 4 psum planes (m x hi/lo) per chunk,
  chunked DMA to DRAM.

Output per core: [128, 512] = scores[b, csub*128 + p, q0 + col//8],
csub = col % 8; host reassembles + final reshape.
"""

import numpy as np

import concourse.bacc as bacc
import concourse.mybir as mybir
import concourse.tile as tile
from concourse.bass_utils import run_bass_kernel_spmd

F32 = mybir.dt.float32
F16 = mybir.dt.float16
TANH = mybir.ActivationFunctionType.Tanh
ADD = mybir.AluOpType.add

B, CTX, QRS, D = 2, 1024, 256, 256
N_CORES = 8
QL = (B * QRS) // N_CORES        # 64 queries per core
UNITS = 2 * QL                   # (m, q) pairs
GS = 8                           # units per S/T tile
NG = UNITS // GS                 # 16 groups
FREE = GS * CTX                  # 8192
# (x_part, w_part) product terms; lo*lo dropped (~2^-22, negligible)
PARTS = [(0, 0), (0, 1), (1, 0)]


def _build_nc():
    nc = bacc.Bacc("TRN2", target_bir_lowering=False, debug=False,
                   enable_asserts=False)

    # fp16 hi/lo pairs, prepared host-side
    ctx_d = [nc.dram_tensor(f"ctx{p}", [D, CTX], F16, kind="ExternalInput")
             for p in range(2)]
    q_d = [nc.dram_tensor(f"q{p}", [D, QL], F16, kind="ExternalInput")
           for p in range(2)]
    wc_d = [nc.dram_tensor(f"wc{p}", [D, D], F16, kind="ExternalInput")
            for p in range(2)]
    wq_d = [nc.dram_tensor(f"wq{p}", [D, D], F16, kind="ExternalInput")
            for p in range(2)]
    bq2 = nc.dram_tensor("bq2", [128, 2], F32, kind="ExternalInput")
    # v interleaved (vh0, vl0, vh1, vl1)
    vs_d = nc.dram_tensor("vs", [128, 4], F16, kind="ExternalInput")
    out = nc.dram_tensor("out", [128, 8 * QL], F32, kind="ExternalOutput")

    with tile.TileContext(nc) as tc:
        with (
            tc.tile_pool(name="consts", bufs=1) as cp,
            tc.tile_pool(name="sp", bufs=3) as sp,
            tc.tile_pool(name="tp", bufs=3) as tp,
            tc.tile_pool(name="pre_ps", bufs=2, space="PSUM") as ppre,
            tc.tile_pool(name="out_ps", bufs=1, space="PSUM") as pout,
        ):
            # ---------- ACT table warmup ----------
            # first Tanh triggers a ~2.7us ACT_TABLE_LOAD; do it on a dummy
            # tile immediately so it overlaps the DMA/prologue instead of
            # delaying the first real tanh
            warm = cp.tile([128, 1], F32, tag="warm", name="warm")
            nc.vector.memset(warm[:], 0.0)
            nc.scalar.activation(warm[:], warm[:], TANH)

            # ---------- load inputs (small/urgent first) ----------
            wc_t = [[cp.tile([128, D], F16, tag=f"wc{p}{k}", name=f"wc{p}{k}")
                     for k in range(2)] for p in range(2)]
            wq_t = [[cp.tile([128, D], F16, tag=f"wqw{p}{k}", name=f"wqw{p}{k}")
                     for k in range(2)] for p in range(2)]
            q_t = [[cp.tile([128, QL], F16, tag=f"qf{p}{k}", name=f"qf{p}{k}")
                    for k in range(2)] for p in range(2)]
            bq_t = cp.tile([128, 2], F32, tag="bq", name="bq")
            vs = cp.tile([128, 4], F16, tag="vs", name="vs")
            nc.sync.dma_start(bq_t[:], bq2[:])
            nc.sync.dma_start(vs[:], vs_d[:])
            for p in range(2):
                for k in range(2):
                    sl = slice(k * 128, (k + 1) * 128)
                    nc.sync.dma_start(wc_t[p][k][:], wc_d[p][sl, :])
                    nc.sync.dma_start(wq_t[p][k][:], wq_d[p][sl, :])
                    nc.sync.dma_start(q_t[p][k][:], q_d[p][sl, :])
            # context, chunked so prologue matmuls can start early
            ctx_t = [[cp.tile([128, CTX], F16, tag=f"ctx{p}{k}",
                              name=f"ctx{p}{k}")
                      for k in range(2)] for p in range(2)]
            for n in range(2):
                nsl = slice(n * 512, (n + 1) * 512)
                for p in range(2):
                    for k in range(2):
                        sl = slice(k * 128, (k + 1) * 128)
                        nc.sync.dma_start(ctx_t[p][k][:, nsl],
                                          ctx_d[p][sl, nsl])

            # ---------- uh = Wq @ qT + bq  (fp32) ----------
            uhT = [cp.tile([128, QL], F32, tag=f"uhT{m}", name=f"uhT{m}")
                   for m in range(2)]
            for m in range(2):
                msl = slice(m * 128, (m + 1) * 128)
                ps_uh = ppre.tile([128, QL], F32, tag="psuh", name=f"psuh{m}")
                first = True
                for xp, wp in PARTS:
                    for k in range(2):
                        nc.tensor.matmul(ps_uh[:], lhsT=wq_t[wp][k][:, msl],
                                         rhs=q_t[xp][k][:],
                                         start=first,
                                         stop=(xp, wp) == PARTS[-1] and k == 1)
                        first = False
                nc.vector.tensor_scalar_add(uhT[m][:], ps_uh[:],
                                            bq_t[:, m:m + 1])

            # ---------- wqT = Wc @ ctxT  (fp16 out for fast DVE adds) ----------
            wqT = [cp.tile([128, CTX], F16, tag=f"wqT{m}", name=f"wqT{m}")
                   for m in range(2)]
            for m in range(2):
                msl = slice(m * 128, (m + 1) * 128)
                for n in range(2):
                    nsl = slice(n * 512, (n + 1) * 512)
                    ps_wq = ppre.tile([128, 512], F32, tag="pswq",
                                      name=f"pswq{m}_{n}")
                    first = True
                    for xp, wp in PARTS:
                        for k in range(2):
                            nc.tensor.matmul(
                                ps_wq[:], lhsT=wc_t[wp][k][:, msl],
                                rhs=ctx_t[xp][k][:, nsl],
                                start=first,
                                stop=(xp, wp) == PARTS[-1] and k == 1)
                            first = False
                    nc.vector.tensor_copy(wqT[m][:, nsl], ps_wq[:])

            # ---------- main loop ----------
            # scoresT psum [128, 2048]: column (q*8+csub)*4 + m*2 + {hi,lo};
            # every matmul its own closed accumulation group (one open group
            # per psum bank is a HW constraint)
            ps_out = pout.tile([128, 4 * 8 * QL], F32, tag="pso", name="pso")
            stage = cp.tile([128, 8 * QL], F32, tag="stage", name="stage")
            pr = ps_out[:].rearrange("p (a b) -> p a b", b=4)
            planes = [pr[:, :, i:i + 1].squeeze(2) for i in range(4)]

            for g in range(NG):
                s = sp.tile([128, FREE], F16, tag="s", name=f"s{g}")
                for j in range(GS):
                    u = g * GS + j
                    m, q = u // QL, u % QL
                    nc.vector.tensor_scalar_add(
                        s[:, j * CTX:(j + 1) * CTX], wqT[m][:],
                        uhT[m][:, q:q + 1])
                t = tp.tile([128, FREE], F16, tag="t", name=f"t{g}")
                nc.scalar.activation(t[:], s[:], TANH)
                for j in range(GS):
                    u = g * GS + j
                    m, q = u // QL, u % QL
                    for csub in range(8):
                        col = 4 * (q * 8 + csub) + 2 * m
                        nc.tensor.matmul(
                            ps_out[:, col:col + 2],
                            lhsT=t[:, j * CTX + csub * 128:
                                   j * CTX + (csub + 1) * 128],
                            rhs=vs[:, 2 * m:2 * m + 2],
                            start=True, stop=True)
                # epilogue chunk: once a group of m=1 units is reduced, its
                # q-range has all 4 planes -> combine + store, overlapped
                # with remaining groups (DVE reads at most one PSUM operand
                # per instruction, hence the chain through stage)
                if g >= NG // 2:
                    q0 = (g - NG // 2) * GS
                    csl = slice(q0 * 8, (q0 + GS) * 8)
                    nc.vector.tensor_copy(stage[:, csl], planes[0][:, csl])
                    for i in range(1, 4):
                        nc.vector.scalar_tensor_tensor(
                            stage[:, csl], planes[i][:, csl], 0.0,
                            stage[:, csl], ADD, ADD)
                    nc.sync.dma_start(out[:, csl], stage[:, csl])

    nc.compile()
    return nc


_NC_CACHE = {}


def _get_nc():
    if "nc" not in _NC_CACHE:
        _NC_CACHE["nc"] = _build_nc()
    return _NC_CACHE["nc"]


def _hilo(x):
    hi = x.astype(np.float16)
    lo = (x - hi.astype(np.float32)).astype(np.float16)
    return np.ascontiguousarray(hi), np.ascontiguousarray(lo)


def _in_maps(context, queries, Wc, Wq, bq, v):
    wc_p = _hilo(Wc.T.astype(np.float32))
    wq_p = _hilo(Wq.T.astype(np.float32))
    bq2 = np.ascontiguousarray(bq.reshape(2, 128).T, dtype=np.float32)
    v2 = v.reshape(2, 128).T.astype(np.float32)    # [128, 2]
    vh = v2.astype(np.float16)
    vl = (v2 - vh.astype(np.float32)).astype(np.float16)
    vs = np.ascontiguousarray(
        np.stack([vh[:, 0], vl[:, 0], vh[:, 1], vl[:, 1]], axis=1))
    ctx_p = [_hilo(context[b].T.astype(np.float32)) for b in range(B)]
    maps = []
    for k in range(N_CORES):
        b = k // (N_CORES // B)
        q0 = (k % (N_CORES // B)) * QL
        q_p = _hilo(queries[b, q0:q0 + QL, :].T.astype(np.float32))
        maps.append({
            "ctx0": ctx_p[b][0], "ctx1": ctx_p[b][1],
            "q0": q_p[0], "q1": q_p[1],
            "wc0": wc_p[0], "wc1": wc_p[1],
            "wq0": wq_p[0], "wq1": wq_p[1],
            "bq2": bq2, "vs": vs,
        })
    return maps


def run(context, queries, Wc, Wq, bq, v, trace=False, **spmd_kwargs):
    nc = _get_nc()
    maps = _in_maps(np.asarray(context), np.asarray(queries), np.asarray(Wc),
                    np.asarray(Wq), np.asarray(bq), np.asarray(v))
    res = run_bass_kernel_spmd(nc, maps, core_ids=list(range(N_CORES)),
                               trace=trace, **spmd_kwargs)
    scores = np.empty((B, CTX, QRS), dtype=np.float32)
    for k in range(N_CORES):
        b = k // (N_CORES // B)
        q0 = (k % (N_CORES // B)) * QL
        arr = res.results[k]["out"]            # [128, QL*8]
        # arr[p, q*8+csub] = scores[b, csub*128+p, q0+q]
        blk = arr.reshape(128, QL, 8).transpose(2, 0, 1).reshape(CTX, QL)
        scores[b, :, q0:q0 + QL] = blk
    return scores.reshape(B, QRS, CTX), res


def kernel(context, queries, Wc, Wq, bq, v):
    out, _ = run(context, queries, Wc, Wq, bq, v, trace=False)
    return out


# revision 12
# speedup vs baseline: 1.0468x; 1.0468x over previous
"""Bahdanau-additive attention scorer on 8 TRN2 NeuronCores.

reference:
  wq = context @ Wc.T            (B, CTX, D)
  uh = queries @ Wq.T + bq       (B, QRS, D)
  scores[b,c,q] = sum_h v[h] * tanh(wq[b,c,h] + uh[b,q,h])
  return scores.reshape(B, QRS, CTX)     # flat view of (B, CTX, QRS)

Sharding: over (batch, query): core k handles batch k//4, queries
(k%4)*64 ... +64, with the full 1024-row context of its batch (context
replicated across the 4 cores of a batch, weights everywhere).

Inputs are pre-split on host into fp16 (hi, lo) pairs so the device matmuls
get ~fp32 accuracy at fp16 PE speed (products keep hi*hi + hi*lo + lo*hi).

Device layout: hidden dim h on partitions (2 tiles of 128 = "m" halves).
  prologue (PE): wqT[h,c] = Wc @ ctx.T (2 x [128,1024], fp16 out)
                 uhT[h,q] = Wq @ q.T + bq (2 x [128,64], fp32)
  main loop over 16 groups of 8 units (unit = (m, q), m-major):
    S[:, j*1024:+1024] = wqT_m + uhT_m[:, q]   DVE tensor_scalar_add (fp16)
    T = tanh(S)  fp16                          ACT, [128, 8192] per instr
    per 128-col chunk: psum[:, col:col+2] = T_chunk.T @ (v_hi|v_lo)  PE N=2
  epilogue (overlapped): DVE sums the 4 psum planes (m x hi/lo) per chunk,
  chunked DMA to DRAM.

Output per core: [128, 512] = scores[b, csub*128 + p, q0 + col//8],
csub = col % 8; host reassembles + final reshape.
"""

import numpy as np

import concourse.bacc as bacc
import concourse.mybir as mybir
import concourse.tile as tile
from concourse.bass_utils import run_bass_kernel_spmd

F32 = mybir.dt.float32
F16 = mybir.dt.float16
TANH = mybir.ActivationFunctionType.Tanh
ADD = mybir.AluOpType.add

B, CTX, QRS, D = 2, 1024, 256, 256
N_CORES = 8
QL = (B * QRS) // N_CORES        # 64 queries per core
UNITS = 2 * QL                   # (m, q) pairs
GS = 8                           # units per S/T tile
NG = UNITS // GS                 # 16 groups
FREE = GS * CTX                  # 8192
# (x_part, w_part) product terms; lo*lo dropped (~2^-22, negligible)
PARTS = [(0, 0), (0, 1), (1, 0)]


def _build_nc():
    nc = bacc.Bacc("TRN2", target_bir_lowering=False, debug=False,
                   enable_asserts=False)

    # fp16 hi/lo pairs, prepared host-side. DMA dispatch costs ~0.6us per
    # descriptor on an engine sequencer, so inputs are packed into few big
    # tensors: wpack = (wc_hi|wc_lo|wq_hi|wq_lo|q_hi|q_lo) columns,
    # small = (bq2 f32 | vs fp16-pairs bitcast to f32).
    WPC = 2 * D + 2 * D + 2 * QL                     # 1152 columns
    wpack_d = nc.dram_tensor("wpack", [D, WPC], F16, kind="ExternalInput")
    ctx_d = [nc.dram_tensor(f"ctx{p}", [D, CTX], F16, kind="ExternalInput")
             for p in range(2)]
    small_d = nc.dram_tensor("small", [128, 4], F32, kind="ExternalInput")
    out = nc.dram_tensor("out", [128, 8 * QL], F32, kind="ExternalOutput")

    with tile.TileContext(nc) as tc:
        with (
            tc.tile_pool(name="consts", bufs=1) as cp,
            tc.tile_pool(name="sp", bufs=3) as sp,
            tc.tile_pool(name="tp", bufs=3) as tp,
            tc.tile_pool(name="pre_ps", bufs=2, space="PSUM") as ppre,
            tc.tile_pool(name="out_ps", bufs=1, space="PSUM") as pout,
        ):
            # ---------- ACT table warmup ----------
            # first Tanh triggers a ~2.7us ACT_TABLE_LOAD; do it on a dummy
            # tile immediately so it overlaps the DMA/prologue instead of
            # delaying the first real tanh
            warm = cp.tile([128, 1], F32, tag="warm", name="warm")
            nc.vector.memset(warm[:], 0.0)
            nc.scalar.activation(warm[:], warm[:], TANH)

            # ---------- load inputs: 7 DMAs spread over engine queues ----------
            small_t = cp.tile([128, 4], F32, tag="small", name="small_t")
            wpk = [cp.tile([128, WPC], F16, tag=f"wp{k}", name=f"wp{k}")
                   for k in range(2)]
            ctx_t = [[cp.tile([128, CTX], F16, tag=f"ctx{p}{k}",
                              name=f"ctx{p}{k}")
                      for k in range(2)] for p in range(2)]
            # HWDGE queues exist only on SP (sync) + ACT (scalar); gpsimd is
            # SWDGE. hi parts (needed first) on the HW queues, lo on gpsimd.
            nc.sync.dma_start(small_t[:], small_d[:])
            nc.sync.dma_start(wpk[0][:], wpack_d[0:128, :])
            nc.scalar.dma_start(wpk[1][:], wpack_d[128:256, :])
            nc.sync.dma_start(ctx_t[0][0][:], ctx_d[0][0:128, :])
            nc.scalar.dma_start(ctx_t[0][1][:], ctx_d[0][128:256, :])
            nc.gpsimd.dma_start(ctx_t[1][0][:], ctx_d[1][0:128, :])
            nc.gpsimd.dma_start(ctx_t[1][1][:], ctx_d[1][128:256, :])
            # views into the packed tiles
            wc_t = [[wpk[k][:, p * D:(p + 1) * D] for k in range(2)]
                    for p in range(2)]
            wq_t = [[wpk[k][:, 2 * D + p * D:2 * D + (p + 1) * D]
                     for k in range(2)] for p in range(2)]
            q_t = [[wpk[k][:, 4 * D + p * QL:4 * D + (p + 1) * QL]
                    for k in range(2)] for p in range(2)]
            bq_t = small_t[:, 0:2]
            vs = small_t[:, 2:4].bitcast(F16)      # [128, 4] fp16

            # ---------- uh = Wq @ qT + bq  (fp32) ----------
            uhT = [cp.tile([128, QL], F32, tag=f"uhT{m}", name=f"uhT{m}")
                   for m in range(2)]
            for m in range(2):
                msl = slice(m * 128, (m + 1) * 128)
                ps_uh = ppre.tile([128, QL], F32, tag="psuh", name=f"psuh{m}")
                first = True
                for xp, wp in PARTS:
                    for k in range(2):
                        nc.tensor.matmul(ps_uh[:], lhsT=wq_t[wp][k][:, msl],
                                         rhs=q_t[xp][k][:],
                                         start=first,
                                         stop=(xp, wp) == PARTS[-1] and k == 1)
                        first = False
                nc.vector.tensor_scalar_add(uhT[m][:], ps_uh[:],
                                            bq_t[:, m:m + 1])

            # ---------- wqT = Wc @ ctxT  (fp16 out for fast DVE adds) ----------
            wqT = [cp.tile([128, CTX], F16, tag=f"wqT{m}", name=f"wqT{m}")
                   for m in range(2)]
            for m in range(2):
                msl = slice(m * 128, (m + 1) * 128)
                for n in range(2):
                    nsl = slice(n * 512, (n + 1) * 512)
                    ps_wq = ppre.tile([128, 512], F32, tag="pswq",
                                      name=f"pswq{m}_{n}")
                    first = True
                    for xp, wp in PARTS:
                        for k in range(2):
                            nc.tensor.matmul(
                                ps_wq[:], lhsT=wc_t[wp][k][:, msl],
                                rhs=ctx_t[xp][k][:, nsl],
                                start=first,
                                stop=(xp, wp) == PARTS[-1] and k == 1)
                            first = False
                    nc.vector.tensor_copy(wqT[m][:, nsl], ps_wq[:])

            # ---------- main loop ----------
            # scoresT psum [128, 2048]: column (q*8+csub)*4 + m*2 + {hi,lo};
            # every matmul its own closed accumulation group (one open group
            # per psum bank is a HW constraint)
            ps_out = pout.tile([128, 4 * 8 * QL], F32, tag="pso", name="pso")
            stage = cp.tile([128, 8 * QL], F32, tag="stage", name="stage")
            pr = ps_out[:].rearrange("p (a b) -> p a b", b=4)
            planes = [pr[:, :, i:i + 1].squeeze(2) for i in range(4)]

            for g in range(NG):
                s = sp.tile([128, FREE], F16, tag="s", name=f"s{g}")
                for j in range(GS):
                    u = g * GS + j
                    m, q = u // QL, u % QL
                    nc.vector.tensor_scalar_add(
                        s[:, j * CTX:(j + 1) * CTX], wqT[m][:],
                        uhT[m][:, q:q + 1])
                t = tp.tile([128, FREE], F16, tag="t", name=f"t{g}")
                nc.scalar.activation(t[:], s[:], TANH)
                for j in range(GS):
                    u = g * GS + j
                    m, q = u // QL, u % QL
                    for csub in range(8):
                        col = 4 * (q * 8 + csub) + 2 * m
                        nc.tensor.matmul(
                            ps_out[:, col:col + 2],
                            lhsT=t[:, j * CTX + csub * 128:
                                   j * CTX + (csub + 1) * 128],
                            rhs=vs[:, 2 * m:2 * m + 2],
                            start=True, stop=True)
                # epilogue chunk: once a group of m=1 units is reduced, its
                # q-range has all 4 planes -> combine + store, overlapped
                # with remaining groups (DVE reads at most one PSUM operand
                # per instruction, hence the chain through stage)
                if g >= NG // 2:
                    q0 = (g - NG // 2) * GS
                    csl = slice(q0 * 8, (q0 + GS) * 8)
                    nc.vector.tensor_copy(stage[:, csl], planes[0][:, csl])
                    for i in range(1, 4):
                        nc.vector.scalar_tensor_tensor(
                            stage[:, csl], planes[i][:, csl], 0.0,
                            stage[:, csl], ADD, ADD)
                    nc.sync.dma_start(out[:, csl], stage[:, csl])

    nc.compile()
    return nc


_NC_CACHE = {}


def _get_nc():
    if "nc" not in _NC_CACHE:
        _NC_CACHE["nc"] = _build_nc()
    return _NC_CACHE["nc"]


def _hilo(x):
    hi = x.astype(np.float16)
    lo = (x - hi.astype(np.float32)).astype(np.float16)
    return np.ascontiguousarray(hi), np.ascontiguousarray(lo)


def _in_maps(context, queries, Wc, Wq, bq, v):
    wc_p = _hilo(Wc.T.astype(np.float32))
    wq_p = _hilo(Wq.T.astype(np.float32))
    bq2 = bq.reshape(2, 128).T.astype(np.float32)  # [128, 2]
    v2 = v.reshape(2, 128).T.astype(np.float32)    # [128, 2]
    vh = v2.astype(np.float16)
    vl = (v2 - vh.astype(np.float32)).astype(np.float16)
    vs = np.stack([vh[:, 0], vl[:, 0], vh[:, 1], vl[:, 1]], axis=1)
    small = np.ascontiguousarray(
        np.concatenate([bq2, vs.view(np.float32)], axis=1))  # [128, 4] f32
    ctx_p = [_hilo(context[b].T.astype(np.float32)) for b in range(B)]
    maps = []
    for k in range(N_CORES):
        b = k // (N_CORES // B)
        q0 = (k % (N_CORES // B)) * QL
        q_p = _hilo(queries[b, q0:q0 + QL, :].T.astype(np.float32))
        wpack = np.ascontiguousarray(np.concatenate(
            [wc_p[0], wc_p[1], wq_p[0], wq_p[1], q_p[0], q_p[1]], axis=1))
        maps.append({
            "wpack": wpack,
            "ctx0": ctx_p[b][0], "ctx1": ctx_p[b][1],
            "small": small,
        })
    return maps


def run(context, queries, Wc, Wq, bq, v, trace=False, **spmd_kwargs):
    nc = _get_nc()
    maps = _in_maps(np.asarray(context), np.asarray(queries), np.asarray(Wc),
                    np.asarray(Wq), np.asarray(bq), np.asarray(v))
    res = run_bass_kernel_spmd(nc, maps, core_ids=list(range(N_CORES)),
                               trace=trace, **spmd_kwargs)
    scores = np.empty((B, CTX, QRS), dtype=np.float32)
    for k in range(N_CORES):
        b = k // (N_CORES // B)
        q0 = (k % (N_CORES // B)) * QL
        arr = res.results[k]["out"]            # [128, QL*8]
        # arr[p, q*8+csub] = scores[b, csub*128+p, q0+q]
        blk = arr.reshape(128, QL, 8).transpose(2, 0, 1).reshape(CTX, QL)
        scores[b, :, q0:q0 + QL] = blk
    return scores.reshape(B, QRS, CTX), res


def kernel(context, queries, Wc, Wq, bq, v):
    out, _ = run(context, queries, Wc, Wq, bq, v, trace=False)
    return out
